# revision 10
# baseline (speedup 1.0000x reference)
"""Trainium2 Bass kernel for nn_DA_conv (dense_cnn).

Model (per batch element b, channel c):
  kern = leaky(d @ kW1.T) @ kW2.T            -> per-(b,c) 3x3 depthwise filter
  dw   = depthwise_conv3x3(x, kern), pad=1   (cross-correlation)
  act  = leaky(dw)
  out  = conv1x1(act, convW) + convB + x * sigmoid-attention(d)

Sharding: data-parallel over batch B=16 across 8 cores (2 images/core).
Per-core layout: 128 SBUF partitions = (2 images x 64 channels).

Strategy (fp8 DoubleRow everywhere on the PE):
- x is split on the host into x_hi = fp8(x) and x_lo = fp8(x - x_hi), both
  uploaded as zero-padded 130x130 planes (fully contiguous DMA, no device
  memsets). The tiny d-MLPs (kern, att) are computed on the host; their
  outputs become fp8 diagonal weight matrices uploaded directly.
- Depthwise: 9 taps = 5 DoubleRow matmuls per 512-px chunk (2 taps per
  matmul via the k-tile dim; moving AP k-tile stride = tap-offset delta).
  x_hi only; kern quantized to fp8 (validated: final rel err ~1.3e-2).
- conv1x1 + attention + residual in 3 DoubleRow matmuls per chunk:
    (convW_hi . a8, convW_res . a8)         stride-0 k-tile on a8
    (att8 . x_hi,   att8 . x_lo)            k-tile stride = plane
    (attres8 . x_hi, 0)                     fp8 residual of sigmoid att
  where a8 = fp8(leaky(dw)) and convW_hi/res is an fp8 hi/lo split.
- ACT does the prelu (psum->fp8), DVE the evacuation (psum->bf16 + bias);
  Pool/gpsimd cannot read PSUM here.
- DMA issue: x planes on SP, weights on ACT, outputs on the gpsimd/Pool
  SWDGE queue (a waiting DMA holds its queue's sequencer, so outputs must
  not share a queue with compute dispatch); the first x slice is small so compute starts
  early; warm-up matmuls on an identity (no DMA dependency) ramp the PE
  p-state while the first DMAs land.
- Output is written bf16 (halves the out-DMA; adds ~4e-4 rel rounding).
"""
import numpy as np
import ml_dtypes

import concourse.bacc as bacc
import concourse.bass as bass
import concourse.mybir as mybir
import concourse.tile as tile
from concourse.bass_utils import run_bass_kernel_spmd
from concourse.masks import make_identity

F32 = mybir.dt.float32
BF16 = mybir.dt.bfloat16
F8 = mybir.dt.float8e4
NF8 = ml_dtypes.float8_e4m3
AF = mybir.ActivationFunctionType
ALU = mybir.AluOpType
PM = mybir.MatmulPerfMode.DoubleRow

B, C, H, W = 16, 64, 128, 128
NCORES = 8
BL = B // NCORES          # images per core (2)
P = BL * C                # partitions used (128)
WP = W + 2                # padded row length (130)
HPAD = H + 2              # padded rows (130)
PLANE = HPAD * WP         # 16900
NEG = 0.1                 # leaky slope

CB_ROWS = 16              # compute band rows
CH_ROWS = 4               # chunk rows (512 psum elements)
NBANDS = H // CB_ROWS     # 8
NCH = CB_ROWS // CH_ROWS  # 4 chunks per band
# padded-row DMA slices (disjoint; first one small so compute starts early)
IN_SLICES = [(0, 12), (12, 34), (34, 56), (56, 78), (78, 100),
             (100, 122), (122, 130)]
NWARM = 50                # PE p-state warm-up matmuls (64 cols each)

# packed fp8 weights blob layout: wdw | attw | cwb8
WDW_O, ATTW_O, CWB_O = 0, 10 * P, 13 * P
WPK_COLS = 15 * P

# depthwise tap pairs: (first tap, second tap); t = 3*(dy+1) + (dx+1)
TAP_PAIRS = [(0, 1), (2, 3), (4, 5), (6, 7), (8, 8)]

_CACHE = {}


def _tap_base(rr, t):
    dy, dx = t // 3 - 1, t % 3 - 1
    return (rr + 1 + dy) * WP + 1 + dx


def _subap(apx, off, dims):
    part = list(list(apx.ap)[0])
    return bass.AP(apx.tensor, apx.offset + off,
                   [part] + [list(d) for d in dims])


def _build():
    nc = bacc.Bacc("TRN2", target_bir_lowering=False, debug=False)

    xhl_d = nc.dram_tensor("xhl", [P, 2 * PLANE], F8, kind="ExternalInput")
    wpk_d = nc.dram_tensor("wpk8", [P, WPK_COLS], F8, kind="ExternalInput")
    cbf_d = nc.dram_tensor("cbf", [P, 1], F32, kind="ExternalInput")
    out_d = nc.dram_tensor("out", [P, H * W], BF16, kind="ExternalOutput")

    with tile.TileContext(nc) as tc:
        with (
            tc.tile_pool(name="consts", bufs=1) as consts,
            tc.tile_pool(name="a8p", bufs=4) as a8p,
            tc.tile_pool(name="psA", bufs=4, space="PSUM") as psA,
            tc.tile_pool(name="psB", bufs=4, space="PSUM") as psB,
        ):
            xhl = consts.tile([P, 2 * PLANE], F8)
            xap = xhl[:, :]
            wpk = consts.tile([P, WPK_COLS], F8)
            cbf = consts.tile([P, 1], F32)
            ident = consts.tile([P, P], BF16)
            outb = consts.tile([P, H * W], BF16)

            # weights first (they gate the first matmul), then x slices.
            # hi slices run one ahead of lo: the depthwise only needs hi,
            # the conv/att pass needs lo slightly later.
            nc.sync.dma_start(out=wpk[:, 0: 10 * P],
                              in_=wpk_d.ap()[:, 0: 10 * P])

            def xslice(q, k):
                (a, b) = IN_SLICES[k]
                nc.sync.dma_start(
                    out=xhl[:, q * PLANE + a * WP: q * PLANE + b * WP],
                    in_=xhl_d.ap()[:, q * PLANE + a * WP:
                                   q * PLANE + b * WP])

            xslice(0, 0)
            nc.sync.dma_start(out=wpk[:, 10 * P: WPK_COLS],
                              in_=wpk_d.ap()[:, 10 * P: WPK_COLS])
            xslice(0, 1)
            xslice(1, 0)
            xslice(1, 1)
            nc.sync.dma_start(out=cbf, in_=cbf_d.ap())
            for k in range(2, len(IN_SLICES)):
                xslice(0, k)
                xslice(1, k - 1)
            xslice(1, len(IN_SLICES) - 1)

            # PE p-state warm-up: bf16 matmuls on the identity, all into one
            # PSUM tile (same-engine WAW, no semaphore gaps), no DMA deps.
            make_identity(nc, ident)
            wps = psB.tile([P, 64], F32, tag="B")
            for _ in range(NWARM):
                nc.tensor.matmul(wps, ident, ident[:, 0:64],
                                 start=True, stop=True)

            # weight APs reused by every chunk
            w_dw = [wpk[:, WDW_O + 256 * i: WDW_O + 256 * (i + 1)]
                    .rearrange("p (k m) -> p k m", k=2) for i in range(5)]
            w_cv = wpk[:, CWB_O: CWB_O + 2 * P].rearrange(
                "p (k m) -> p k m", k=2)
            w_at = _subap(wpk[:, :], ATTW_O, [(0, 2), (1, P)])
            w_ar = wpk[:, ATTW_O + P: ATTW_O + 3 * P].rearrange(
                "p (k m) -> p k m", k=2)

            for band in range(NBANDS):
                a8 = a8p.tile([P, CB_ROWS * W], F8, tag="a8")
                for j in range(NCH):
                    rr = band * CB_ROWS + j * CH_ROWS
                    # ---- depthwise: 5 DoubleRow tap-pair matmuls ----
                    psa = psA.tile([P, 512], F32, tag="A")
                    psa3 = psa.rearrange("p (r c) -> p r c", c=128)
                    for i, (ta, tb) in enumerate(TAP_PAIRS):
                        base = _tap_base(rr, ta)
                        s = _tap_base(rr, tb) - base
                        mv = _subap(xap, base,
                                    [(s, 2), (WP, CH_ROWS), (1, W)])
                        nc.tensor.matmul(psa3, w_dw[i], mv,
                                         start=(i == 0), stop=(i == 4),
                                         perf_mode=PM)
                    # ---- leaky -> fp8 (ACT) ----
                    nc.scalar.activation(a8[:, j * 512: (j + 1) * 512], psa,
                                         AF.Prelu, alpha=NEG)

                    # ---- conv1x1 + attention + residual ----
                    psb = psB.tile([P, 512], F32, tag="B")
                    psb3 = psb.rearrange("p (r c) -> p r c", c=128)
                    nc.tensor.matmul(psb, w_cv,
                                     _subap(a8[:, :], j * 512,
                                            [(0, 2), (1, 512)]),
                                     start=True, stop=False, perf_mode=PM)
                    abase = (rr + 1) * WP + 1
                    nc.tensor.matmul(psb3, w_at,
                                     _subap(xap, abase,
                                            [(PLANE, 2), (WP, CH_ROWS),
                                             (1, W)]),
                                     start=False, stop=False, perf_mode=PM)
                    nc.tensor.matmul(psb3, w_ar,
                                     _subap(xap, abase,
                                            [(0, 2), (WP, CH_ROWS), (1, W)]),
                                     start=False, stop=True, perf_mode=PM)
                    # ---- psum -> bf16 out (+convB) on DVE ----
                    # (the very last chunk splits across DVE+ACT so the
                    # drain chain after the final matmul is shorter)
                    if band == NBANDS - 1 and j == NCH - 1:
                        nc.vector.tensor_scalar(
                            outb[:, rr * W: rr * W + 256],
                            psb[:, 0:256], cbf[:, 0:1], None, ALU.add)
                        nc.scalar.activation(
                            outb[:, rr * W + 256: rr * W + 512],
                            psb[:, 256:512], AF.Identity, bias=cbf[:, 0:1])
                    else:
                        nc.vector.tensor_scalar(
                            outb[:, rr * W: rr * W + 512],
                            psb, cbf[:, 0:1], None, ALU.add)
                    # output DMAs ride the Pool (SWDGE) and SP queues —
                    # a waiting DMA blocks its queue's sequencer, so they
                    # must not share a queue with compute dispatch. The
                    # last band issues per-chunk DMAs to shorten the tail.
                    if band == NBANDS - 1:
                        o0 = rr * W
                        eng = (nc.sync, nc.gpsimd, nc.sync,
                               nc.scalar)[j]
                        eng.dma_start(out=out_d.ap()[:, o0: o0 + 512],
                                      in_=outb[:, o0: o0 + 512])
                    elif j % 2 == 1:
                        o0 = (rr - CH_ROWS) * W
                        eng = nc.gpsimd if (band * NCH + j) % 4 == 1 \
                            else nc.sync
                        eng.dma_start(
                            out=out_d.ap()[:, o0: o0 + 1024],
                            in_=outb[:, o0: o0 + 1024])

    nc.compile()
    return nc


def _leaky_np(v):
    return np.where(v >= 0, v, NEG * v)


def kernel(x, d, kW1, kW2, convW, convB, caW1, caW2, _trace=False):
    x = np.asarray(x, np.float32)
    d = np.asarray(d, np.float32)
    kW1 = np.asarray(kW1, np.float32)
    kW2 = np.asarray(kW2, np.float32)
    convW = np.asarray(convW, np.float32)
    convB = np.asarray(convB, np.float32)
    caW1 = np.asarray(caW1, np.float32)
    caW2 = np.asarray(caW2, np.float32)
    if "nc" not in _CACHE:
        _CACHE["nc"] = _build()
    nc = _CACHE["nc"]

    # tiny per-sample MLPs on host: kern [B, C, 9], att [B, Cout]
    kern = (_leaky_np(d @ kW1.T) @ kW2.T).reshape(B, C, 9)
    att = 1.0 / (1.0 + np.exp(-(_leaky_np(d @ caW1.T) @ caW2.T)))

    cw8 = convW.astype(NF8).astype(np.float32)
    cwres8 = (convW - cw8).astype(NF8)
    cwb8 = np.zeros((P, 2 * P), NF8)
    for bi in range(BL):
        sl = slice(bi * C, (bi + 1) * C)
        cwb8[sl, bi * C:(bi + 1) * C] = cw8.T.astype(NF8)
        cwb8[sl, P + bi * C: P + (bi + 1) * C] = cwres8.T
    cbf = np.tile(convB, BL)[:, None].astype(np.float32)

    xh = x.astype(NF8)
    xl = (x - xh.astype(np.float32)).astype(NF8)
    k8 = kern.astype(NF8)
    at8 = att.astype(NF8)
    atres8 = (att - at8.astype(np.float32)).astype(NF8)

    in_maps = []
    rng = np.arange(P)
    for c in range(NCORES):
        sl = slice(c * BL, (c + 1) * BL)
        xhl = np.zeros((P, 2, HPAD, WP), NF8)
        xhl[:, 0, 1:H + 1, 1:W + 1] = xh[sl].reshape(P, H, W)
        xhl[:, 1, 1:H + 1, 1:W + 1] = xl[sl].reshape(P, H, W)

        wpk = np.zeros((P, WPK_COLS), NF8)
        kc = k8[sl].reshape(P, 9)
        for t in range(9):
            wpk[rng, WDW_O + t * P + rng] = kc[:, t]
        wpk[rng, ATTW_O + rng] = at8[sl].reshape(P)
        wpk[rng, ATTW_O + P + rng] = atres8[sl].reshape(P)
        wpk[:, CWB_O: CWB_O + 2 * P] = cwb8

        in_maps.append({
            "xhl": np.ascontiguousarray(xhl.reshape(P, 2 * PLANE)),
            "wpk8": wpk,
            "cbf": cbf,
        })

    last_err = None
    for _attempt in range(3):
        try:
            res = run_bass_kernel_spmd(nc, in_maps,
                                       core_ids=list(range(NCORES)),
                                       trace=_trace)
            break
        except Exception as e:  # transient NRT device errors recover on retry
            last_err = e
    else:
        raise last_err
    out = np.concatenate(
        [r["out"].astype(np.float32).reshape(BL, C, H, W)
         for r in res.results], axis=0)
    if _trace:
        return out, res
    return out
